# revision 1
# baseline (speedup 1.0000x reference)
"""BitwiseMLP Trainium2 kernel: 8-way data-parallel over the batch dim.

Math (per reference):
  h0 = x @ W0.T + b0; h0 = BN0(h0); s0 = sign(h0)
  h1 = s0 @ sign(W1).T + b1; h1 = BN1(h1); s1 = sign(h1)
  out = (s1 @ sign(W2).T + b2) * out_scale

Device strategy (per core, batch shard of 8192 rows; activations stay
transposed [channel, batch] end-to-end so the device does zero transposes):
  - L0 = x@W0.T at fp32-class accuracy via a 3-matmul split:
      x = xh + xl, W0 = Wh + Wl with xh/Wh rounded to 12-bit significands
      (read exactly by the PE's float32r/fp22 operand path);
      h0 = xh@Wh (f32r) + xh@Wl + xl@Wh (both fp16 at full rate).
    The fp16 correction operands are pre-scaled by 2^+-6 on host (net scale
    1 per product, exact powers of two) to escape fp16's subnormal range;
    all rounding lands at ~2^-24 relative. The dropped xl@Wl term is
    ~2^-24 relative as well, so L0 matches a native-fp32 matmul's accuracy.
  - BN+sign fuse into one ScalarE activation per tile:
    s = Sign(psum * A + B) with per-channel A/B, output fp8e4 (+-1 exact).
  - L1/L2 are exact +-1 fp8e4 matmuls with DoubleRow (2x rate); results are
    small even integers accumulated exactly in fp32 PSUM.
  - Final eviction: Identity activation out = psum*out_scale + b2*out_scale.
  - Startup: W0 weight DMAs are split per output-channel block and
    deadline-ordered behind the first batch-tile's x DMAs (bt0 runs all
    f32r mains before its corrections) so the PE starts ~16us in instead
    of waiting for all 14MB of weights.
Host does the batch shard, the transposes and the hi/lo splits; the output
comes back transposed per core and is re-assembled in numpy.
"""
import os
import sys
import types

import numpy as np
import ml_dtypes

import concourse.bass as bass
import concourse.mybir as mybir
import concourse.tile as tile
from concourse import bacc
from concourse.bass_utils import run_bass_kernel_spmd


def _ensure_axon_hooks():
    """concourse.bass_utils imports antenv.axon_hooks when tracing is
    requested (BASS_TRACE=1). The trimmed image lacks that module, which
    would turn an optional profile into a crash — synthesize it, wiring the
    real NTFF hook when libaxon_pjrt.so is present."""
    try:
        import antenv.axon_hooks  # noqa: F401
        return
    except ImportError:
        pass
    try:
        import antenv
    except ImportError:
        return
    mod = types.ModuleType("antenv.axon_hooks")
    state = {"hook": None}
    mod.set_axon_ntff_profile_hook = lambda h: state.update(hook=h)
    mod.get_axon_ntff_profile_hook = lambda: state["hook"]
    sys.modules["antenv.axon_hooks"] = mod
    antenv.axon_hooks = mod
    so = "/opt/axon/libaxon_pjrt.so"
    if os.path.exists(so):
        try:
            from trn_agent_boot.trn_boot import _ntff_profile_via_ctypes
            mod.set_axon_ntff_profile_hook(_ntff_profile_via_ctypes(so))
            import concourse.bass_utils as _bu
            _real_upload = _bu.upload_artifacts

            def _safe_upload(tmpdir):
                try:
                    return _real_upload(tmpdir)
                except Exception:
                    return f"local:{tmpdir}"

            _bu.upload_artifacts = _safe_upload
        except Exception:
            pass


_ensure_axon_hooks()

dt = mybir.dt
P = 128
D = 1024
B = 65536
NCORES = 8
BS = B // NCORES          # 8192 batch rows per core
BT = 512                  # batch-tile width (columns of transposed activations)
NBT = BS // BT            # 16 batch tiles per core
KO = D // P               # 8 k-subtiles of 128 channels
EPS = 1e-5
CSCALE = np.float32(64.0)  # 2^6 correction-term balance scale

LAST_RESULTS = None       # BassKernelResults of the most recent run (for profiling)
_NC = None                # cached compiled Bass module (build once per process)


def _round_sig12(a: np.ndarray) -> np.ndarray:
    """Round fp32 magnitudes to 12-bit significands (11 explicit mantissa
    bits), round-half-to-even. Values of this form pass through the PE's
    float32r (fp22) operand read exactly."""
    u = a.view(np.uint32).astype(np.uint64)
    half = np.uint64(1 << 11)
    one = np.uint64(1)
    r = (u + half - one + ((u >> np.uint64(12)) & one)) & ~np.uint64((1 << 12) - 1)
    return r.astype(np.uint32).view(np.float32)


def _build():
    nc = bacc.Bacc(num_devices=NCORES)
    xh = nc.dram_tensor("xh", [P, KO, BS], dt.float32r, kind="ExternalInput")
    xhs = nc.dram_tensor("xhs", [P, KO, BS], dt.float16, kind="ExternalInput")
    xls = nc.dram_tensor("xls", [P, KO, BS], dt.float16, kind="ExternalInput")
    w0h = nc.dram_tensor("w0h", [P, KO, D], dt.float32r, kind="ExternalInput")
    w0hs = nc.dram_tensor("w0hs", [P, KO, D], dt.float16, kind="ExternalInput")
    w0ls = nc.dram_tensor("w0ls", [P, KO, D], dt.float16, kind="ExternalInput")
    w1 = nc.dram_tensor("w1", [P, KO, D], dt.float8e4, kind="ExternalInput")
    w2 = nc.dram_tensor("w2", [P, KO, D], dt.float8e4, kind="ExternalInput")
    vec = nc.dram_tensor("vec", [P, 6, KO], dt.float32, kind="ExternalInput")
    out = nc.dram_tensor("out", [P, KO, BS], dt.float32, kind="ExternalOutput")

    Sign = mybir.ActivationFunctionType.Sign
    Ident = mybir.ActivationFunctionType.Identity
    DR = mybir.MatmulPerfMode.DoubleRow
    ts = bass.ts

    with tile.TileContext(nc) as tc:
        with (
            tc.tile_pool(name="wpool", bufs=1) as wpool,
            tc.tile_pool(name="xpool", bufs=2) as xpool,
            tc.tile_pool(name="spool", bufs=2) as spool,
            tc.tile_pool(name="opool", bufs=3) as opool,
            tc.tile_pool(name="pspool", bufs=8, space="PSUM") as pspool,
        ):
            w0h_sb = wpool.tile([P, KO, D], dt.float32r)
            w0hs_sb = wpool.tile([P, KO, D], dt.float16)
            w0ls_sb = wpool.tile([P, KO, D], dt.float16)
            w1_sb = wpool.tile([P, KO, D], dt.float8e4)
            w2_sb = wpool.tile([P, KO, D], dt.float8e4)
            vec_sb = wpool.tile([P, 6, KO], dt.float32)

            xh_t, xhs_t, xls_t = xh[:], xhs[:], xls[:]
            w0h_t, w0hs_t, w0ls_t = w0h[:], w0hs[:], w0ls[:]
            out_t = out[:]

            # bt0's x tiles first so the PE can start early; W0 chunks are
            # split per output-channel block m and follow in consumption order.
            sl0 = bass.ds(0, BT)
            xh_sb0 = xpool.tile([P, KO, BT], dt.float32r, tag="xh")
            xhs_sb0 = xpool.tile([P, KO, BT], dt.float16, tag="xhs")
            xls_sb0 = xpool.tile([P, KO, BT], dt.float16, tag="xls")
            # arrival order mirrors consumption: xh + f32r weights feed the
            # bt0 mains pass; the fp16 correction tensors follow.
            # deadline-ordered arrivals: mains m0/m1 first, then the
            # correction tensors for m0, then the rest interleaved.
            nc.sync.dma_start(xh_sb0, xh_t[:, :, sl0])
            nc.sync.dma_start(w0h_sb[:, :, ts(0, P)], w0h_t[:, :, ts(0, P)])
            nc.sync.dma_start(w0h_sb[:, :, ts(1, P)], w0h_t[:, :, ts(1, P)])
            nc.sync.dma_start(w0h_sb[:, :, ts(2, P)], w0h_t[:, :, ts(2, P)])
            nc.sync.dma_start(w0h_sb[:, :, ts(3, P)], w0h_t[:, :, ts(3, P)])
            nc.sync.dma_start(xhs_sb0, xhs_t[:, :, sl0])
            nc.sync.dma_start(xls_sb0, xls_t[:, :, sl0])
            for m in range(4, KO):
                msl = ts(m, P)
                nc.sync.dma_start(w0h_sb[:, :, msl], w0h_t[:, :, msl])
            for m in range(KO):
                msl = ts(m, P)
                nc.sync.dma_start(w0hs_sb[:, :, msl], w0hs_t[:, :, msl])
                nc.sync.dma_start(w0ls_sb[:, :, msl], w0ls_t[:, :, msl])
            nc.sync.dma_start(w1_sb, w1[:])
            nc.sync.dma_start(w2_sb, w2[:])
            nc.sync.dma_start(vec_sb, vec[:])

            for bt in range(NBT):
                sl = bass.ds(bt * BT, BT)
                if bt == 0:
                    xh_sb, xhs_sb, xls_sb = xh_sb0, xhs_sb0, xls_sb0
                else:
                    xh_sb = xpool.tile([P, KO, BT], dt.float32r, tag="xh")
                    xhs_sb = xpool.tile([P, KO, BT], dt.float16, tag="xhs")
                    xls_sb = xpool.tile([P, KO, BT], dt.float16, tag="xls")
                    nc.sync.dma_start(xh_sb, xh_t[:, :, sl])
                    nc.sync.dma_start(xhs_sb, xhs_t[:, :, sl])
                    nc.sync.dma_start(xls_sb, xls_t[:, :, sl])

                # ---- L0: f32r main + fp16 corrections, fused BN0+sign ----
                s0_sb = spool.tile([P, KO, BT], dt.float8e4, tag="s0")
                if bt == 0:
                    # startup special case: run all mains first so the PE only
                    # waits on xh + w0h; correction DMAs land meanwhile.
                    ps_l = [pspool.tile([P, BT], dt.float32, tag="ps",
                                        name=f"ps0_{i}")
                            for i in range(KO)]
                    for m in range(KO):
                        for k in range(KO):
                            nc.tensor.matmul(ps_l[m], w0h_sb[:, k, ts(m, P)],
                                             xh_sb[:, k, :],
                                             start=k == 0, stop=False)
                    for m in range(KO):
                        for k in range(KO):
                            nc.tensor.matmul(ps_l[m], w0ls_sb[:, k, ts(m, P)],
                                             xhs_sb[:, k, :],
                                             start=False, stop=False)
                            nc.tensor.matmul(ps_l[m], w0hs_sb[:, k, ts(m, P)],
                                             xls_sb[:, k, :],
                                             start=False, stop=k == KO - 1)
                        nc.scalar.activation(s0_sb[:, m, :], ps_l[m], Sign,
                                             bias=vec_sb[:, 1, m:m + 1],
                                             scale=vec_sb[:, 0, m:m + 1])
                else:
                    for m in range(KO):
                        ps = pspool.tile([P, BT], dt.float32, tag="ps")
                        for k in range(KO):
                            nc.tensor.matmul(ps, w0h_sb[:, k, ts(m, P)],
                                             xh_sb[:, k, :],
                                             start=k == 0, stop=False)
                        for k in range(KO):
                            nc.tensor.matmul(ps, w0ls_sb[:, k, ts(m, P)],
                                             xhs_sb[:, k, :],
                                             start=False, stop=False)
                            nc.tensor.matmul(ps, w0hs_sb[:, k, ts(m, P)],
                                             xls_sb[:, k, :],
                                             start=False, stop=k == KO - 1)
                        nc.scalar.activation(s0_sb[:, m, :], ps, Sign,
                                             bias=vec_sb[:, 1, m:m + 1],
                                             scale=vec_sb[:, 0, m:m + 1])

                # ---- L1: fp8 +-1 DoubleRow matmuls, fused BN1+sign ----
                s1_sb = spool.tile([P, KO, BT], dt.float8e4, tag="s1")
                for m in range(KO):
                    ps = pspool.tile([P, BT], dt.float32, tag="ps")
                    for kp in range(KO // 2):
                        nc.tensor.matmul(ps, w1_sb[:, 2 * kp:2 * kp + 2, ts(m, P)],
                                         s0_sb[:, 2 * kp:2 * kp + 2, :],
                                         start=kp == 0, stop=kp == KO // 2 - 1,
                                         perf_mode=DR)
                    nc.scalar.activation(s1_sb[:, m, :], ps, Sign,
                                         bias=vec_sb[:, 3, m:m + 1],
                                         scale=vec_sb[:, 2, m:m + 1])

                # ---- L2: fp8 +-1 DoubleRow matmuls, fused scale+bias ----
                for m in range(KO):
                    ps = pspool.tile([P, BT], dt.float32, tag="ps")
                    for kp in range(KO // 2):
                        nc.tensor.matmul(ps, w2_sb[:, 2 * kp:2 * kp + 2, ts(m, P)],
                                         s1_sb[:, 2 * kp:2 * kp + 2, :],
                                         start=kp == 0, stop=kp == KO // 2 - 1,
                                         perf_mode=DR)
                    o_sb = opool.tile([P, BT], dt.float32, tag="om")
                    nc.scalar.activation(o_sb, ps, Ident,
                                         bias=vec_sb[:, 5, m:m + 1],
                                         scale=vec_sb[:, 4, m:m + 1])
                    nc.sync.dma_start(out_t[:, m, sl], o_sb)

    nc.compile()
    return nc


def kernel(**inputs) -> np.ndarray:
    global LAST_RESULTS
    f32 = np.float32
    f16 = np.float16
    x = np.asarray(inputs["x"], f32)
    W0 = np.asarray(inputs["W0"], f32)
    b0 = np.asarray(inputs["b0"], f32)
    W1 = np.asarray(inputs["W1"], f32)
    b1 = np.asarray(inputs["b1"], f32)
    W2 = np.asarray(inputs["W2"], f32)
    b2 = np.asarray(inputs["b2"], f32)
    bn0_g = np.asarray(inputs["bn0_g"], f32)
    bn0_b = np.asarray(inputs["bn0_b"], f32)
    bn0_rm = np.asarray(inputs["bn0_rm"], f32)
    bn0_rv = np.asarray(inputs["bn0_rv"], f32)
    bn1_g = np.asarray(inputs["bn1_g"], f32)
    bn1_b = np.asarray(inputs["bn1_b"], f32)
    bn1_rm = np.asarray(inputs["bn1_rm"], f32)
    bn1_rv = np.asarray(inputs["bn1_rv"], f32)
    osc = np.asarray(inputs["out_scale"], f32)

    # per-channel affine folds (BN in eval mode):
    #   bn0(h+b0) = h*A0 + B0 ; bn1(h+b1) = h*A1 + B1 ; out = h*CS + CB
    inv0 = (bn0_g / np.sqrt(bn0_rv + EPS)).astype(f32)
    inv1 = (bn1_g / np.sqrt(bn1_rv + EPS)).astype(f32)
    A0, B0 = inv0, ((b0 - bn0_rm) * inv0 + bn0_b).astype(f32)
    A1, B1 = inv1, ((b1 - bn1_rm) * inv1 + bn1_b).astype(f32)
    CS, CB = osc, (b2 * osc).astype(f32)
    vec = np.stack([A0, B0, A1, B1, CS, CB])           # [6, D]
    vec_host = np.ascontiguousarray(
        vec.reshape(6, KO, P).transpose(2, 0, 1))      # [P, 6, KO]

    def pm(a):
        # [cols, D] -> partition-major [P, KO, cols]
        return np.ascontiguousarray(a.T.reshape(KO, P, -1).transpose(1, 0, 2))

    W0h = _round_sig12(W0)
    w0h_host = pm(W0h)                              # f32r main
    w0hs_host = pm((W0h / CSCALE).astype(f16))      # fp16, 2^-6
    w0ls_host = pm(((W0 - W0h) * CSCALE).astype(f16))  # fp16, 2^+6
    e4m3 = mybir.dt.np(dt.float8e4)
    w1_host = pm(np.sign(W1).astype(e4m3))
    w2_host = pm(np.sign(W2).astype(e4m3))

    xh_full = _round_sig12(x)
    xhT = pm(xh_full)                               # [P, KO, B] f32r
    xhsT = pm((xh_full / CSCALE).astype(f16))
    xlsT = pm(((x - xh_full) * CSCALE).astype(f16))

    shared = {
        "w0h": w0h_host, "w0hs": w0hs_host, "w0ls": w0ls_host,
        "w1": w1_host, "w2": w2_host, "vec": vec_host,
    }
    in_maps = []
    for c in range(NCORES):
        bs = slice(c * BS, (c + 1) * BS)
        in_maps.append({
            **shared,
            "xh": np.ascontiguousarray(xhT[:, :, bs]),
            "xhs": np.ascontiguousarray(xhsT[:, :, bs]),
            "xls": np.ascontiguousarray(xlsT[:, :, bs]),
        })

    global _NC
    if _NC is None:
        _NC = _build()
    res = run_bass_kernel_spmd(_NC, in_maps, core_ids=list(range(NCORES)))
    LAST_RESULTS = res

    out = np.empty((B, D), f32)
    for c in range(NCORES):
        # [P, KO, BS] -> [BS, KO*P] with channel = ko*P + p
        o = res.results[c]["out"].transpose(2, 1, 0).reshape(BS, D)
        out[c * BS:(c + 1) * BS] = o
    return out



# revision 2
# speedup vs baseline: 1.3077x; 1.3077x over previous
"""BitwiseMLP Trainium2 kernel: 8-way data-parallel over the batch dim.

Math (per reference):
  h0 = x @ W0.T + b0; h0 = BN0(h0); s0 = sign(h0)
  h1 = s0 @ sign(W1).T + b1; h1 = BN1(h1); s1 = sign(h1)
  out = (s1 @ sign(W2).T + b2) * out_scale

Device strategy (per core, batch shard of 8192 rows; activations stay
transposed [channel, batch] end-to-end so the device does zero transposes):
  - L0 = x@W0.T to ~2^-17.5 accuracy via a main pass + one fp8 chain:
      xh = rne12(x), W12 = rne12(W0)  (the PE's f32r operand read rounds
      to 12-bit significands RNE — verified on HW by probe);
      main: xh @ (W12 * 2^29) as f32r, exact;
      corrections: xl@W12 + x@Wr  (xl = x-xh, Wr = W0-W12) as ONE
      fp8e4 DoubleRow chain of K=2048: slots [e4m3(xl*2^17) @ e4m3(W*2^12)]
      and [e4m3(x*2^5) @ e4m3(Wr*2^24)].  Every product lands at scale
      2^29, so all 16 MMs accumulate into one PSUM; the 2^-29 folds into
      the BN scale of the Sign activation.  Predicted (and numpy-simulated)
      end-to-end rel err ~1.2e-2 vs the fp32 reference — the few sign
      flips this induces near |h0|~1e-6 dominate the error budget.
  - BN+sign fuse into one ScalarE activation per tile:
    s = Sign(psum * A + B) with per-channel A/B, output fp8e4 (+-1 exact).
  - L1/L2 are exact +-1 fp8e4 matmuls with DoubleRow (2x rate); results are
    small even integers accumulated exactly in fp32 PSUM.
  - Final eviction: Identity activation out = psum*out_scale + b2*out_scale.
  - Startup: W0 weight DMAs are split per output-channel block and
    deadline-ordered behind the first batch-tile's x DMAs (bt0 runs all
    f32r mains before its corrections) so the PE starts early instead
    of waiting for all weights.
Host does the batch shard, the transposes and the hi/lo splits; the output
comes back transposed per core and is re-assembled in numpy.
"""
import os
import sys
import types

import numpy as np
import ml_dtypes

import concourse.bass as bass
import concourse.mybir as mybir
import concourse.tile as tile
from concourse import bacc
from concourse.bass_utils import run_bass_kernel_spmd


def _ensure_axon_hooks():
    """concourse.bass_utils imports antenv.axon_hooks when tracing is
    requested (BASS_TRACE=1). The trimmed image lacks that module, which
    would turn an optional profile into a crash — synthesize it, wiring the
    real NTFF hook when libaxon_pjrt.so is present."""
    try:
        import antenv.axon_hooks  # noqa: F401
        return
    except ImportError:
        pass
    try:
        import antenv
    except ImportError:
        return
    mod = types.ModuleType("antenv.axon_hooks")
    state = {"hook": None}
    mod.set_axon_ntff_profile_hook = lambda h: state.update(hook=h)
    mod.get_axon_ntff_profile_hook = lambda: state["hook"]
    sys.modules["antenv.axon_hooks"] = mod
    antenv.axon_hooks = mod
    so = "/opt/axon/libaxon_pjrt.so"
    if os.path.exists(so):
        try:
            from trn_agent_boot.trn_boot import _ntff_profile_via_ctypes
            mod.set_axon_ntff_profile_hook(_ntff_profile_via_ctypes(so))
            import concourse.bass_utils as _bu
            _real_upload = _bu.upload_artifacts

            def _safe_upload(tmpdir):
                try:
                    return _real_upload(tmpdir)
                except Exception:
                    return f"local:{tmpdir}"

            _bu.upload_artifacts = _safe_upload
        except Exception:
            pass


_ensure_axon_hooks()

dt = mybir.dt
P = 128
D = 1024
B = 65536
NCORES = 8
BS = B // NCORES          # 8192 batch rows per core
BT = 512                  # batch-tile width (columns of transposed activations)
NBT = BS // BT            # 16 batch tiles per core
KO = D // P               # 8 k-subtiles of 128 channels
NQ = 2 * KO               # fp8 correction slots: [xl*2^17 (8) | x*2^5 (8)]
EPS = 1e-5
PSC = np.float32(2.0**29)  # uniform product scale of the L0 PSUM

LAST_RESULTS = None       # BassKernelResults of the most recent run (for profiling)
_NC = None                # cached compiled Bass module (build once per process)


def _round_sig12(a: np.ndarray) -> np.ndarray:
    """Round fp32 magnitudes to 12-bit significands (11 explicit mantissa
    bits), round-half-to-even — exactly the PE's f32r operand read."""
    u = a.view(np.uint32).astype(np.uint64)
    half = np.uint64(1 << 11)
    one = np.uint64(1)
    r = (u + half - one + ((u >> np.uint64(12)) & one)) & ~np.uint64((1 << 12) - 1)
    return r.astype(np.uint32).view(np.float32)


def _build():
    nc = bacc.Bacc(num_devices=NCORES)
    xh = nc.dram_tensor("xh", [P, KO, BS], dt.float32r, kind="ExternalInput")
    xq = nc.dram_tensor("xq", [P, NQ, BS], dt.float8e4, kind="ExternalInput")
    w0h = nc.dram_tensor("w0h", [P, KO, D], dt.float32r, kind="ExternalInput")
    wq = nc.dram_tensor("wq", [P, NQ, D], dt.float8e4, kind="ExternalInput")
    w1 = nc.dram_tensor("w1", [P, KO, D], dt.float8e4, kind="ExternalInput")
    w2 = nc.dram_tensor("w2", [P, KO, D], dt.float8e4, kind="ExternalInput")
    vec = nc.dram_tensor("vec", [P, 6, KO], dt.float32, kind="ExternalInput")
    out = nc.dram_tensor("out", [P, KO, BS], dt.float32, kind="ExternalOutput")

    Sign = mybir.ActivationFunctionType.Sign
    Ident = mybir.ActivationFunctionType.Identity
    DR = mybir.MatmulPerfMode.DoubleRow
    ts = bass.ts

    with tile.TileContext(nc) as tc:
        with (
            tc.tile_pool(name="wpool", bufs=1) as wpool,
            tc.tile_pool(name="xpool", bufs=2) as xpool,
            tc.tile_pool(name="spool", bufs=2) as spool,
            tc.tile_pool(name="opool", bufs=3) as opool,
            tc.tile_pool(name="pspool", bufs=8, space="PSUM") as pspool,
        ):
            w0h_sb = wpool.tile([P, KO, D], dt.float32r)
            wq_sb = wpool.tile([P, NQ, D], dt.float8e4)
            w1_sb = wpool.tile([P, KO, D], dt.float8e4)
            w2_sb = wpool.tile([P, KO, D], dt.float8e4)
            vec_sb = wpool.tile([P, 6, KO], dt.float32)

            xh_t, xq_t, w0h_t, wq_t = xh[:], xq[:], w0h[:], wq[:]
            out_t = out[:]

            # bt0's x tiles first so the PE can start early; W0 chunks are
            # split per output-channel block m and follow in consumption order.
            sl0 = bass.ds(0, BT)
            xh_sb0 = xpool.tile([P, KO, BT], dt.float32r, tag="xh")
            xq_sb0 = xpool.tile([P, NQ, BT], dt.float8e4, tag="xq")
            # deadline-ordered arrivals: f32r weights for the bt0 mains first,
            # then the fp8 correction tensors.
            nc.sync.dma_start(xh_sb0, xh_t[:, :, sl0])
            nc.sync.dma_start(w0h_sb[:, :, ts(0, P)], w0h_t[:, :, ts(0, P)])
            nc.sync.dma_start(w0h_sb[:, :, ts(1, P)], w0h_t[:, :, ts(1, P)])
            nc.sync.dma_start(w0h_sb[:, :, ts(2, P)], w0h_t[:, :, ts(2, P)])
            nc.sync.dma_start(w0h_sb[:, :, ts(3, P)], w0h_t[:, :, ts(3, P)])
            nc.sync.dma_start(xq_sb0, xq_t[:, :, sl0])
            for m in range(4, KO):
                msl = ts(m, P)
                nc.sync.dma_start(w0h_sb[:, :, msl], w0h_t[:, :, msl])
            for m in range(KO):
                msl = ts(m, P)
                nc.sync.dma_start(wq_sb[:, :, msl], wq_t[:, :, msl])
            nc.sync.dma_start(w1_sb, w1[:])
            nc.sync.dma_start(w2_sb, w2[:])
            nc.sync.dma_start(vec_sb, vec[:])

            for bt in range(NBT):
                sl = bass.ds(bt * BT, BT)
                if bt == 0:
                    xh_sb, xq_sb = xh_sb0, xq_sb0
                else:
                    xh_sb = xpool.tile([P, KO, BT], dt.float32r, tag="xh")
                    xq_sb = xpool.tile([P, NQ, BT], dt.float8e4, tag="xq")
                    nc.sync.dma_start(xh_sb, xh_t[:, :, sl])
                    nc.sync.dma_start(xq_sb, xq_t[:, :, sl])

                # ---- L0: f32r main + fp8 DR corrections, fused BN0+sign ----
                s0_sb = spool.tile([P, KO, BT], dt.float8e4, tag="s0")
                if bt == 0:
                    # startup special case: run all mains first so the PE only
                    # waits on xh + w0h; correction DMAs land meanwhile.
                    ps_l = [pspool.tile([P, BT], dt.float32, tag="ps",
                                        name=f"ps0_{i}")
                            for i in range(KO)]
                    for m in range(KO):
                        for k in range(KO):
                            nc.tensor.matmul(ps_l[m], w0h_sb[:, k, ts(m, P)],
                                             xh_sb[:, k, :],
                                             start=k == 0, stop=False)
                    for m in range(KO):
                        for i in range(KO):
                            nc.tensor.matmul(ps_l[m],
                                             wq_sb[:, 2 * i:2 * i + 2, ts(m, P)],
                                             xq_sb[:, 2 * i:2 * i + 2, :],
                                             start=False, stop=i == KO - 1,
                                             perf_mode=DR)
                        nc.scalar.activation(s0_sb[:, m, :], ps_l[m], Sign,
                                             bias=vec_sb[:, 1, m:m + 1],
                                             scale=vec_sb[:, 0, m:m + 1])
                else:
                    for m in range(KO):
                        ps = pspool.tile([P, BT], dt.float32, tag="ps")
                        for k in range(KO):
                            nc.tensor.matmul(ps, w0h_sb[:, k, ts(m, P)],
                                             xh_sb[:, k, :],
                                             start=k == 0, stop=False)
                        for i in range(KO):
                            nc.tensor.matmul(ps,
                                             wq_sb[:, 2 * i:2 * i + 2, ts(m, P)],
                                             xq_sb[:, 2 * i:2 * i + 2, :],
                                             start=False, stop=i == KO - 1,
                                             perf_mode=DR)
                        nc.scalar.activation(s0_sb[:, m, :], ps, Sign,
                                             bias=vec_sb[:, 1, m:m + 1],
                                             scale=vec_sb[:, 0, m:m + 1])

                # ---- L1: fp8 +-1 DoubleRow matmuls, fused BN1+sign ----
                s1_sb = spool.tile([P, KO, BT], dt.float8e4, tag="s1")
                for m in range(KO):
                    ps = pspool.tile([P, BT], dt.float32, tag="ps")
                    for kp in range(KO // 2):
                        nc.tensor.matmul(ps, w1_sb[:, 2 * kp:2 * kp + 2, ts(m, P)],
                                         s0_sb[:, 2 * kp:2 * kp + 2, :],
                                         start=kp == 0, stop=kp == KO // 2 - 1,
                                         perf_mode=DR)
                    nc.scalar.activation(s1_sb[:, m, :], ps, Sign,
                                         bias=vec_sb[:, 3, m:m + 1],
                                         scale=vec_sb[:, 2, m:m + 1])

                # ---- L2: fp8 +-1 DoubleRow matmuls, fused scale+bias ----
                for m in range(KO):
                    ps = pspool.tile([P, BT], dt.float32, tag="ps")
                    for kp in range(KO // 2):
                        nc.tensor.matmul(ps, w2_sb[:, 2 * kp:2 * kp + 2, ts(m, P)],
                                         s1_sb[:, 2 * kp:2 * kp + 2, :],
                                         start=kp == 0, stop=kp == KO // 2 - 1,
                                         perf_mode=DR)
                    o_sb = opool.tile([P, BT], dt.float32, tag="om")
                    nc.scalar.activation(o_sb, ps, Ident,
                                         bias=vec_sb[:, 5, m:m + 1],
                                         scale=vec_sb[:, 4, m:m + 1])
                    nc.sync.dma_start(out_t[:, m, sl], o_sb)

    nc.compile()
    return nc


def kernel(**inputs) -> np.ndarray:
    global LAST_RESULTS
    f32 = np.float32
    x = np.asarray(inputs["x"], f32)
    W0 = np.asarray(inputs["W0"], f32)
    b0 = np.asarray(inputs["b0"], f32)
    W1 = np.asarray(inputs["W1"], f32)
    b1 = np.asarray(inputs["b1"], f32)
    W2 = np.asarray(inputs["W2"], f32)
    b2 = np.asarray(inputs["b2"], f32)
    bn0_g = np.asarray(inputs["bn0_g"], f32)
    bn0_b = np.asarray(inputs["bn0_b"], f32)
    bn0_rm = np.asarray(inputs["bn0_rm"], f32)
    bn0_rv = np.asarray(inputs["bn0_rv"], f32)
    bn1_g = np.asarray(inputs["bn1_g"], f32)
    bn1_b = np.asarray(inputs["bn1_b"], f32)
    bn1_rm = np.asarray(inputs["bn1_rm"], f32)
    bn1_rv = np.asarray(inputs["bn1_rv"], f32)
    osc = np.asarray(inputs["out_scale"], f32)

    # per-channel affine folds (BN in eval mode):
    #   bn0(h+b0) = h*A0 + B0 ; bn1(h+b1) = h*A1 + B1 ; out = h*CS + CB
    # L0's psum carries h*2^29, so A0 absorbs the 2^-29.
    inv0 = (bn0_g / np.sqrt(bn0_rv + EPS)).astype(f32)
    inv1 = (bn1_g / np.sqrt(bn1_rv + EPS)).astype(f32)
    A0, B0 = (inv0 / PSC).astype(f32), ((b0 - bn0_rm) * inv0 + bn0_b).astype(f32)
    A1, B1 = inv1, ((b1 - bn1_rm) * inv1 + bn1_b).astype(f32)
    CS, CB = osc, (b2 * osc).astype(f32)
    vec = np.stack([A0, B0, A1, B1, CS, CB])           # [6, D]
    vec_host = np.ascontiguousarray(
        vec.reshape(6, KO, P).transpose(2, 0, 1))      # [P, 6, KO]

    def pm(a):
        # [cols, D] -> partition-major [P, KO, cols]
        return np.ascontiguousarray(a.T.reshape(KO, P, -1).transpose(1, 0, 2))

    e4 = mybir.dt.np(dt.float8e4)
    W12 = _round_sig12(W0)
    Wr = W0 - W12
    w0h_host = pm(W12 * PSC)                         # f32r main, scale 2^29
    wq_full = np.concatenate([                       # [2D, D] -> pm -> [P,2KO,D]
        (W0 * np.float32(2.0**12)).astype(e4).reshape(D, D),
        (Wr * np.float32(2.0**24)).astype(e4).reshape(D, D),
    ], axis=1)  # concat along the input-channel axis: [D, 2D]
    wq_host = np.ascontiguousarray(
        wq_full.T.reshape(2 * KO, P, D).transpose(1, 0, 2))
    w1_host = pm(np.sign(W1).astype(e4))
    w2_host = pm(np.sign(W2).astype(e4))

    xh_full = _round_sig12(x)
    xl = x - xh_full
    xhT = pm(xh_full)                                # [P, KO, B] f32r
    xqT = np.concatenate([
        pm((xl * np.float32(2.0**17)).astype(e4)),
        pm((x * np.float32(2.0**5)).astype(e4)),
    ], axis=1)                                       # [P, 2KO, B] fp8

    shared = {
        "w0h": w0h_host, "wq": wq_host,
        "w1": w1_host, "w2": w2_host, "vec": vec_host,
    }
    in_maps = []
    for c in range(NCORES):
        bs = slice(c * BS, (c + 1) * BS)
        in_maps.append({
            **shared,
            "xh": np.ascontiguousarray(xhT[:, :, bs]),
            "xq": np.ascontiguousarray(xqT[:, :, bs]),
        })

    global _NC
    if _NC is None:
        _NC = _build()
    res = run_bass_kernel_spmd(_NC, in_maps, core_ids=list(range(NCORES)))
    LAST_RESULTS = res

    out = np.empty((B, D), f32)
    for c in range(NCORES):
        # [P, KO, BS] -> [BS, KO*P] with channel = ko*P + p
        o = res.results[c]["out"].transpose(2, 1, 0).reshape(BS, D)
        out[c * BS:(c + 1) * BS] = o
    return out


# revision 5
# speedup vs baseline: 1.3286x; 1.0160x over previous
"""BitwiseMLP Trainium2 kernel: 8-way data-parallel over the batch dim.

Math (per reference):
  h0 = x @ W0.T + b0; h0 = BN0(h0); s0 = sign(h0)
  h1 = s0 @ sign(W1).T + b1; h1 = BN1(h1); s1 = sign(h1)
  out = (s1 @ sign(W2).T + b2) * out_scale

Device strategy (per core, batch shard of 8192 rows; activations stay
transposed [channel, batch] end-to-end so the device does zero transposes):
  - L0 = x@W0.T to ~2^-17.5 accuracy via a main pass + one fp8 chain:
      xh = rne12(x), W12 = rne12(W0)  (the PE's f32r operand read rounds
      to 12-bit significands RNE — verified on HW by probe);
      main: xh @ (W12 * 2^29) as f32r, exact;
      corrections: xl@W12 + x@Wr  (xl = x-xh, Wr = W0-W12) as ONE
      fp8e4 DoubleRow chain of K=2048: slots [e4m3(xl*2^17) @ e4m3(W*2^12)]
      and [e4m3(x*2^5) @ e4m3(Wr*2^24)].  Every product lands at scale
      2^29, so all 16 MMs accumulate into one PSUM; the 2^-29 folds into
      the BN scale of the Sign activation.  Predicted (and numpy-simulated)
      end-to-end rel err ~1.2e-2 vs the fp32 reference — the few sign
      flips this induces near |h0|~1e-6 dominate the error budget.
  - BN+sign fuse into one ScalarE activation per tile:
    s = Sign(psum * A + B) with per-channel A/B, output fp8e4 (+-1 exact).
  - L1/L2 are exact +-1 fp8e4 matmuls with DoubleRow (2x rate); results are
    small even integers accumulated exactly in fp32 PSUM.
  - Final eviction: Identity activation out = psum*out_scale + b2*out_scale.
  - Startup: W0 weight DMAs are split per output-channel block and
    deadline-ordered behind the first batch-tile's x DMAs (bt0 runs all
    f32r mains before its corrections) so the PE starts early instead
    of waiting for all weights.
Host does the batch shard, the transposes and the hi/lo splits; the output
comes back transposed per core and is re-assembled in numpy.
"""
import os
import sys
import types

import numpy as np
import ml_dtypes

import concourse.bass as bass
import concourse.mybir as mybir
import concourse.tile as tile
from concourse import bacc
from concourse.bass_utils import run_bass_kernel_spmd


def _ensure_axon_hooks():
    """concourse.bass_utils imports antenv.axon_hooks when tracing is
    requested (BASS_TRACE=1). The trimmed image lacks that module, which
    would turn an optional profile into a crash — synthesize it, wiring the
    real NTFF hook when libaxon_pjrt.so is present."""
    try:
        import antenv.axon_hooks  # noqa: F401
        return
    except ImportError:
        pass
    try:
        import antenv
    except ImportError:
        return
    mod = types.ModuleType("antenv.axon_hooks")
    state = {"hook": None}
    mod.set_axon_ntff_profile_hook = lambda h: state.update(hook=h)
    mod.get_axon_ntff_profile_hook = lambda: state["hook"]
    sys.modules["antenv.axon_hooks"] = mod
    antenv.axon_hooks = mod
    so = "/opt/axon/libaxon_pjrt.so"
    if os.path.exists(so):
        try:
            from trn_agent_boot.trn_boot import _ntff_profile_via_ctypes
            mod.set_axon_ntff_profile_hook(_ntff_profile_via_ctypes(so))
            import concourse.bass_utils as _bu
            _real_upload = _bu.upload_artifacts

            def _safe_upload(tmpdir):
                try:
                    return _real_upload(tmpdir)
                except Exception:
                    return f"local:{tmpdir}"

            _bu.upload_artifacts = _safe_upload
        except Exception:
            pass


_ensure_axon_hooks()

dt = mybir.dt
P = 128
D = 1024
B = 65536
NCORES = 8
BS = B // NCORES          # 8192 batch rows per core
BT = 512                  # batch-tile width (columns of transposed activations)
NBT = BS // BT            # 16 batch tiles per core
KO = D // P               # 8 k-subtiles of 128 channels
NQ = 2 * KO               # fp8 correction slots: [xl*2^17 (8) | x*2^5 (8)]
EPS = 1e-5
PSC = np.float32(2.0**29)  # uniform product scale of the L0 PSUM

LAST_RESULTS = None       # BassKernelResults of the most recent run (for profiling)
_NC = None                # cached compiled Bass module (build once per process)


def _round_sig12(a: np.ndarray) -> np.ndarray:
    """Round fp32 magnitudes to 12-bit significands (11 explicit mantissa
    bits), round-half-to-even — exactly the PE's f32r operand read."""
    u = a.view(np.uint32).astype(np.uint64)
    half = np.uint64(1 << 11)
    one = np.uint64(1)
    r = (u + half - one + ((u >> np.uint64(12)) & one)) & ~np.uint64((1 << 12) - 1)
    return r.astype(np.uint32).view(np.float32)


def _build():
    nc = bacc.Bacc(num_devices=NCORES)
    xh = nc.dram_tensor("xh", [P, KO, BS], dt.float32r, kind="ExternalInput")
    xq = nc.dram_tensor("xq", [P, NQ, BS], dt.float8e4, kind="ExternalInput")
    w0h = nc.dram_tensor("w0h", [P, KO, D], dt.float32r, kind="ExternalInput")
    wq = nc.dram_tensor("wq", [P, NQ, D], dt.float8e4, kind="ExternalInput")
    w1 = nc.dram_tensor("w1", [P, KO, D], dt.float8e4, kind="ExternalInput")
    w2 = nc.dram_tensor("w2", [P, KO, D], dt.float8e4, kind="ExternalInput")
    vec = nc.dram_tensor("vec", [P, 6, KO], dt.float32, kind="ExternalInput")
    out = nc.dram_tensor("out", [P, KO, BS], dt.float32, kind="ExternalOutput")

    Sign = mybir.ActivationFunctionType.Sign
    Ident = mybir.ActivationFunctionType.Identity
    DR = mybir.MatmulPerfMode.DoubleRow
    ts = bass.ts

    with tile.TileContext(nc) as tc:
        with (
            tc.tile_pool(name="wpool", bufs=1) as wpool,
            tc.tile_pool(name="xpool", bufs=2) as xpool,
            tc.tile_pool(name="spool", bufs=2) as spool,
            tc.tile_pool(name="opool", bufs=3) as opool,
            tc.tile_pool(name="pspool", bufs=8, space="PSUM") as pspool,
        ):
            w0h_sb = wpool.tile([P, KO, D], dt.float32r)
            wq_sb = wpool.tile([P, NQ, D], dt.float8e4)
            w1_sb = wpool.tile([P, KO, D], dt.float8e4)
            w2_sb = wpool.tile([P, KO, D], dt.float8e4)
            vec_sb = wpool.tile([P, 6, KO], dt.float32)

            xh_t, xq_t, w0h_t, wq_t = xh[:], xq[:], w0h[:], wq[:]
            out_t = out[:]

            # Startup is DMA-bound: chains run corrections (fp8, 3 MiB of
            # operands) before mains (f32r, 6 MiB), and the DMA issue order
            # matches consumption so the PE starts ~5us in and never stalls.
            sl0 = bass.ds(0, BT)
            xh_sb0 = xpool.tile([P, KO, BT], dt.float32r, tag="xh")
            xq_sb0 = xpool.tile([P, NQ, BT], dt.float8e4, tag="xq")
            nc.sync.dma_start(vec_sb, vec[:])
            nc.sync.dma_start(xq_sb0, xq_t[:, :, sl0])
            for m in range(KO):
                msl = ts(m, P)
                nc.sync.dma_start(wq_sb[:, :, msl], wq_t[:, :, msl])
            nc.sync.dma_start(xh_sb0, xh_t[:, :, sl0])
            for m in range(KO):
                msl = ts(m, P)
                nc.sync.dma_start(w0h_sb[:, :, msl], w0h_t[:, :, msl])
            nc.sync.dma_start(w1_sb, w1[:])
            nc.sync.dma_start(w2_sb, w2[:])

            for bt in range(NBT):
                sl = bass.ds(bt * BT, BT)
                if bt == 0:
                    xh_sb, xq_sb = xh_sb0, xq_sb0
                else:
                    xh_sb = xpool.tile([P, KO, BT], dt.float32r, tag="xh")
                    xq_sb = xpool.tile([P, NQ, BT], dt.float8e4, tag="xq")
                    nc.sync.dma_start(xq_sb, xq_t[:, :, sl])
                    nc.sync.dma_start(xh_sb, xh_t[:, :, sl])

                # ---- L0: fp8 DR corrections + f32r main, fused BN0+sign ----
                s0_sb = spool.tile([P, KO, BT], dt.float8e4, tag="s0")
                if bt == 0:
                    # startup special case: all correction chains first (they
                    # only need the small fp8 tensors); main DMAs land behind.
                    ps_l = [pspool.tile([P, BT], dt.float32, tag="ps",
                                        name=f"ps0_{i}")
                            for i in range(KO)]
                    for m in range(KO):
                        for i in range(KO):
                            nc.tensor.matmul(ps_l[m],
                                             wq_sb[:, 2 * i:2 * i + 2, ts(m, P)],
                                             xq_sb[:, 2 * i:2 * i + 2, :],
                                             start=i == 0, stop=False,
                                             perf_mode=DR)
                    for m in range(KO):
                        for k in range(KO):
                            nc.tensor.matmul(ps_l[m], w0h_sb[:, k, ts(m, P)],
                                             xh_sb[:, k, :],
                                             start=False, stop=k == KO - 1)
                        nc.scalar.activation(s0_sb[:, m, :], ps_l[m], Sign,
                                             bias=vec_sb[:, 1, m:m + 1],
                                             scale=vec_sb[:, 0, m:m + 1])
                else:
                    for m in range(KO):
                        ps = pspool.tile([P, BT], dt.float32, tag="ps")
                        for i in range(KO):
                            nc.tensor.matmul(ps,
                                             wq_sb[:, 2 * i:2 * i + 2, ts(m, P)],
                                             xq_sb[:, 2 * i:2 * i + 2, :],
                                             start=i == 0, stop=False,
                                             perf_mode=DR)
                        for k in range(KO):
                            nc.tensor.matmul(ps, w0h_sb[:, k, ts(m, P)],
                                             xh_sb[:, k, :],
                                             start=False, stop=k == KO - 1)
                        nc.scalar.activation(s0_sb[:, m, :], ps, Sign,
                                             bias=vec_sb[:, 1, m:m + 1],
                                             scale=vec_sb[:, 0, m:m + 1])

                # ---- L1: fp8 +-1 DoubleRow matmuls, fused BN1+sign ----
                s1_sb = spool.tile([P, KO, BT], dt.float8e4, tag="s1")
                for m in range(KO):
                    ps = pspool.tile([P, BT], dt.float32, tag="ps")
                    for kp in range(KO // 2):
                        nc.tensor.matmul(ps, w1_sb[:, 2 * kp:2 * kp + 2, ts(m, P)],
                                         s0_sb[:, 2 * kp:2 * kp + 2, :],
                                         start=kp == 0, stop=kp == KO // 2 - 1,
                                         perf_mode=DR)
                    nc.scalar.activation(s1_sb[:, m, :], ps, Sign,
                                         bias=vec_sb[:, 3, m:m + 1],
                                         scale=vec_sb[:, 2, m:m + 1])

                # ---- L2: fp8 +-1 DoubleRow matmuls, fused scale+bias ----
                for m in range(KO):
                    ps = pspool.tile([P, BT], dt.float32, tag="ps")
                    for kp in range(KO // 2):
                        nc.tensor.matmul(ps, w2_sb[:, 2 * kp:2 * kp + 2, ts(m, P)],
                                         s1_sb[:, 2 * kp:2 * kp + 2, :],
                                         start=kp == 0, stop=kp == KO // 2 - 1,
                                         perf_mode=DR)
                    o_sb = opool.tile([P, BT], dt.float32, tag="om")
                    nc.scalar.activation(o_sb, ps, Ident,
                                         bias=vec_sb[:, 5, m:m + 1],
                                         scale=vec_sb[:, 4, m:m + 1])
                    nc.sync.dma_start(out_t[:, m, sl], o_sb)

    nc.compile()
    return nc


def kernel(**inputs) -> np.ndarray:
    global LAST_RESULTS
    f32 = np.float32
    x = np.asarray(inputs["x"], f32)
    W0 = np.asarray(inputs["W0"], f32)
    b0 = np.asarray(inputs["b0"], f32)
    W1 = np.asarray(inputs["W1"], f32)
    b1 = np.asarray(inputs["b1"], f32)
    W2 = np.asarray(inputs["W2"], f32)
    b2 = np.asarray(inputs["b2"], f32)
    bn0_g = np.asarray(inputs["bn0_g"], f32)
    bn0_b = np.asarray(inputs["bn0_b"], f32)
    bn0_rm = np.asarray(inputs["bn0_rm"], f32)
    bn0_rv = np.asarray(inputs["bn0_rv"], f32)
    bn1_g = np.asarray(inputs["bn1_g"], f32)
    bn1_b = np.asarray(inputs["bn1_b"], f32)
    bn1_rm = np.asarray(inputs["bn1_rm"], f32)
    bn1_rv = np.asarray(inputs["bn1_rv"], f32)
    osc = np.asarray(inputs["out_scale"], f32)

    # per-channel affine folds (BN in eval mode):
    #   bn0(h+b0) = h*A0 + B0 ; bn1(h+b1) = h*A1 + B1 ; out = h*CS + CB
    # L0's psum carries h*2^29, so A0 absorbs the 2^-29.
    inv0 = (bn0_g / np.sqrt(bn0_rv + EPS)).astype(f32)
    inv1 = (bn1_g / np.sqrt(bn1_rv + EPS)).astype(f32)
    A0, B0 = (inv0 / PSC).astype(f32), ((b0 - bn0_rm) * inv0 + bn0_b).astype(f32)
    A1, B1 = inv1, ((b1 - bn1_rm) * inv1 + bn1_b).astype(f32)
    CS, CB = osc, (b2 * osc).astype(f32)
    vec = np.stack([A0, B0, A1, B1, CS, CB])           # [6, D]
    vec_host = np.ascontiguousarray(
        vec.reshape(6, KO, P).transpose(2, 0, 1))      # [P, 6, KO]

    def pm(a):
        # [cols, D] -> partition-major [P, KO, cols]
        return np.ascontiguousarray(a.T.reshape(KO, P, -1).transpose(1, 0, 2))

    e4 = mybir.dt.np(dt.float8e4)
    W12 = _round_sig12(W0)
    Wr = W0 - W12
    w0h_host = pm(W12 * PSC)                         # f32r main, scale 2^29
    wq_full = np.concatenate([                       # [2D, D] -> pm -> [P,2KO,D]
        (W0 * np.float32(2.0**12)).astype(e4).reshape(D, D),
        (Wr * np.float32(2.0**24)).astype(e4).reshape(D, D),
    ], axis=1)  # concat along the input-channel axis: [D, 2D]
    wq_host = np.ascontiguousarray(
        wq_full.T.reshape(2 * KO, P, D).transpose(1, 0, 2))
    w1_host = pm(np.sign(W1).astype(e4))
    w2_host = pm(np.sign(W2).astype(e4))

    xh_full = _round_sig12(x)
    xl = x - xh_full
    xhT = pm(xh_full)                                # [P, KO, B] f32r
    xqT = np.concatenate([
        pm((xl * np.float32(2.0**17)).astype(e4)),
        pm((x * np.float32(2.0**5)).astype(e4)),
    ], axis=1)                                       # [P, 2KO, B] fp8

    shared = {
        "w0h": w0h_host, "wq": wq_host,
        "w1": w1_host, "w2": w2_host, "vec": vec_host,
    }
    in_maps = []
    for c in range(NCORES):
        bs = slice(c * BS, (c + 1) * BS)
        in_maps.append({
            **shared,
            "xh": np.ascontiguousarray(xhT[:, :, bs]),
            "xq": np.ascontiguousarray(xqT[:, :, bs]),
        })

    global _NC
    if _NC is None:
        _NC = _build()
    res = run_bass_kernel_spmd(_NC, in_maps, core_ids=list(range(NCORES)))
    LAST_RESULTS = res

    out = np.empty((B, D), f32)
    for c in range(NCORES):
        # [P, KO, BS] -> [BS, KO*P] with channel = ko*P + p
        o = res.results[c]["out"].transpose(2, 1, 0).reshape(BS, D)
        out[c * BS:(c + 1) * BS] = o
    return out
